# revision 23
# baseline (speedup 1.0000x reference)
"""CCMLite kernel for Trainium2: GroupNorm(affine=False) + low-rank channel mix.

out = x_norm + u @ (v^T @ x_norm) + shift, with x_norm = groupnorm(x).

Sharding: data-parallel over batch B=16 across 8 cores (2 batch elems/core).
No collectives needed.

x is shipped to the device as fp16 (host-side cast): GroupNorm statistics
from fp16 inputs are accurate to ~1e-5 relative (population stats over 32k
elements average out the rounding), the matmul datapath wants fp16 anyway
(fp32 matmuls cost 2 PE passes), and it halves the HBM read traffic —
per-core HBM becomes 4.2 MB in + 8.4 MB out (f32).

Device-side algebra (per batch element):
  per-channel stats (bn_stats/bn_aggr on DVE, fp16 in / f32 stats)
  group stats via mask matmuls (PE)      -> group mean m_g, rstd s_g
  broadcast to channels via maskT matmul (PE)
  vs[c,r]  = v[c,r] * s_c
  kvec[r]  = sum_c vs[c,r] * m_c
  cst_c    = shift_c - m_c*s_c - sum_r u[c,r]*kvec[r]
  vtx[r,n] = sum_c vs[c,r] * x[c,n]          (PE fp16, K=128 x2, N=512)
  psum[c,n] = sum_r u[c,r]*vtx[r,n] + s_c*x[c,n]   (PE fp16: u^T mm + diag mm)
  out[c,n] = psum[c,n] + cst_c               (ACT Identity copy w/ bias AP,
                                              or DVE tensor_scalar add)
The per-channel constant rides the PSUM->SBUF copy as a per-partition bias,
so no transposes / ones-rows are needed and u^T is a pure DMA input.
"""

from contextlib import ExitStack

import numpy as np

import concourse.bass as bass
import concourse.tile as tile
from concourse import bacc, mybir
from concourse.bass_utils import run_bass_kernel_spmd

N_CORES = 8
B, C, H, W = 16, 256, 64, 64
HW = H * W            # 4096
R = 12                # low rank
G = 32                # groups
GPC = C // G          # 8 channels per group
P = 128               # partitions
CB = C // P           # 2 channel blocks
BPC = B // N_CORES    # 2 batch elements per core
EPS = 1e-6
F32 = mybir.dt.float32
F16 = mybir.dt.float16

_MULT = mybir.AluOpType.mult
_ADD = mybir.AluOpType.add


def build_nc(dve_out_mod=4):
    """Per-core Bass program. dve_out_mod: stage-B output chunk k goes
    through DVE (tensor_scalar add) instead of ACT when k % mod == 0;
    0 -> all chunks on ACT."""
    nc = bacc.Bacc(None, target_bir_lowering=False)
    x_d = nc.dram_tensor("x", [BPC, C, HW], F16, kind="ExternalInput")
    ut_d = nc.dram_tensor("ut", [BPC, CB, R, P], F16, kind="ExternalInput")
    v_d = nc.dram_tensor("v", [BPC, CB, P, R], F32, kind="ExternalInput")
    shift_d = nc.dram_tensor("shift", [BPC, CB, P, 1], F32, kind="ExternalInput")
    gmask_d = nc.dram_tensor("gmask", [P, 16], F32, kind="ExternalInput")
    gmaskT_d = nc.dram_tensor("gmaskT", [16, P], F32, kind="ExternalInput")
    ident_d = nc.dram_tensor("ident16", [P, P], F16, kind="ExternalInput")
    out_d = nc.dram_tensor("out", [BPC, C, HW], F32, kind="ExternalOutput")

    with tile.TileContext(nc) as tc, ExitStack() as ctx:
        consts = ctx.enter_context(tc.tile_pool(name="consts", bufs=1))
        xbp = ctx.enter_context(tc.tile_pool(name="xbp", bufs=8))
        outp = ctx.enter_context(tc.tile_pool(name="outp", bufs=6))
        smalls = ctx.enter_context(tc.tile_pool(name="smalls", bufs=4))
        vaugp = ctx.enter_context(tc.tile_pool(name="vaugp", bufs=2))
        ps_small = ctx.enter_context(
            tc.tile_pool(name="ps_small", bufs=2, space="PSUM"))
        ps_vtx = ctx.enter_context(tc.tile_pool(name="ps_vtx", bufs=1, space="PSUM"))
        ps_out = ctx.enter_context(tc.tile_pool(name="ps_out", bufs=2, space="PSUM"))

        ident_h = consts.tile([P, P], F16)
        nc.gpsimd.dma_start(out=ident_h, in_=ident_d[:, :])
        gmask = consts.tile([P, 16], F32)
        nc.gpsimd.dma_start(out=gmask, in_=gmask_d[:, :])
        gmaskT = consts.tile([16, P], F32)
        nc.gpsimd.dma_start(out=gmaskT, in_=gmaskT_d[:, :])
        eps_t = consts.tile([16, 1], F32)
        nc.vector.memset(eps_t, EPS)
        # persistent vtx rhs tiles, rewritten per batch
        vaugs = []
        for i in range(2):
            va = consts.tile([R, HW], F16, tag=f"vaug{i}")
            vaugs.append(va)

        for b in range(BPC):
            # ---- load x (fp16): 4 tiles of [128, 2048], all on the sync
            # HWDGE ring (FIFO -> in-order completion, stats chase tiles) ----
            xbt = {}
            for cb in range(CB):
                for h in range(2):
                    tb = xbp.tile([P, 2048], F16, tag="xbt")
                    nc.sync.dma_start(
                        out=tb,
                        in_=x_d[b, cb * P:(cb + 1) * P, h * 2048:(h + 1) * 2048],
                    )
                    xbt[(cb, h)] = tb

            # ---- per-channel + group stats, per cb so cb0's chain
            # overlaps cb1's bn_stats on DVE ----
            sm = smalls.tile([P, 4], F32, tag="sm")  # cols [s0, m0, s1, m1]
            gvals = smalls.tile([16, 4], F32, tag="gvals")
            tmpg = smalls.tile([16, 4], F32, tag="tmpg")
            e2 = smalls.tile([P, CB], F32, tag="e2")
            for cb in range(CB):
                st = smalls.tile([P, 8, 6], F32, tag=f"bstats{cb}")
                for j in range(8):
                    nc.vector.bn_stats(
                        out=st[:, j:j + 1, :],
                        in_=xbt[(cb, j // 4)][:, (j % 4) * 512:(j % 4 + 1) * 512],
                    )
                mv = smalls.tile([P, 2], F32, tag=f"mv{cb}")
                nc.vector.bn_aggr(out=mv, in_=st)
                # E[x^2] = mean*mean + var in one DVE op
                nc.vector.scalar_tensor_tensor(
                    out=e2[:, cb:cb + 1], in0=mv[:, 0:1], scalar=mv[:, 0:1],
                    in1=mv[:, 1:2], op0=_MULT, op1=_ADD)
                # group sums (same lhsT for both mms)
                gsum = ps_small.tile([16, 2], F32, tag="ps")
                nc.tensor.matmul(
                    gsum[:, 0:1], lhsT=gmask, rhs=mv[:, 0:1],
                    start=True, stop=True)
                nc.tensor.matmul(
                    gsum[:, 1:2], lhsT=gmask, rhs=e2[:, cb:cb + 1],
                    start=True, stop=True)
                gmean = gvals[:, 2 * cb + 1:2 * cb + 2]
                nc.vector.tensor_scalar_mul(
                    out=gmean, in0=gsum[:, 0:1], scalar1=1.0 / GPC)
                g2 = tmpg[:, 2 * cb:2 * cb + 1]
                nc.vector.tensor_mul(out=g2, in0=gmean, in1=gmean)
                gvar = tmpg[:, 2 * cb + 1:2 * cb + 2]
                # var = E2/8 - mean^2 in one DVE op
                nc.vector.scalar_tensor_tensor(
                    out=gvar, in0=gsum[:, 1:2], scalar=1.0 / GPC,
                    in1=g2, op0=_MULT, op1=mybir.AluOpType.subtract)
                # std = sqrt(var + eps)
                nc.scalar.activation(
                    out=gvar, in_=gvar,
                    func=mybir.ActivationFunctionType.Sqrt, bias=eps_t[:, 0:1],
                    scale=1.0)
                nc.vector.reciprocal(out=gvals[:, 2 * cb:2 * cb + 1], in_=gvar)
                # broadcast this cb's [rstd, mean] to its 128 channels
                bc = ps_small.tile([P, 2], F32, tag="ps")
                nc.tensor.matmul(
                    bc, lhsT=gmaskT, rhs=gvals[:, 2 * cb:2 * cb + 2],
                    start=True, stop=True)
                nc.vector.tensor_copy(out=sm[:, 2 * cb:2 * cb + 2], in_=bc)

            # ---- per-cb small prep ----
            vss, diags, auglist, csts = [], [], [], []
            kvec = ps_small.tile([R, 1], F32, tag="ps")
            for cb in range(CB):
                s_ap = sm[:, 2 * cb:2 * cb + 1]
                vt = smalls.tile([P, R], F32, tag=f"vt{cb}")
                nc.gpsimd.dma_start(out=vt, in_=v_d[b, cb])
                # f32 copy for the (tiny, f32) kvec matmul
                vsf = smalls.tile([P, R], F32, tag=f"vsf{cb}")
                nc.vector.tensor_scalar_mul(out=vsf, in0=vt, scalar1=s_ap)
                # fp16 copy as stage-A lhsT
                vs = smalls.tile([P, R], F16, tag=f"vs{cb}")
                nc.vector.tensor_copy(out=vs, in_=vsf)
                diag = smalls.tile([P, P], F16, tag=f"diag{cb}")
                nc.vector.tensor_scalar_mul(out=diag, in0=ident_h, scalar1=s_ap)
                nc.tensor.matmul(
                    kvec, lhsT=vsf, rhs=sm[:, 2 * cb + 1:2 * cb + 2],
                    start=(cb == 0), stop=(cb == CB - 1))
                aug = smalls.tile([R, P], F16, tag=f"aug{cb}")
                nc.gpsimd.dma_start(out=aug, in_=ut_d[b, cb])
                vss.append(vs)
                diags.append(diag)
                auglist.append(aug)
            kvs = smalls.tile([R, 1], F16, tag="kvs")
            nc.vector.tensor_copy(out=kvs, in_=kvec)

            for cb in range(CB):
                s_ap = sm[:, 2 * cb:2 * cb + 1]
                m_ap = sm[:, 2 * cb + 1:2 * cb + 2]
                ukv = ps_small.tile([P, 1], F32, tag="ps")
                nc.tensor.matmul(ukv, lhsT=auglist[cb], rhs=kvs,
                                 start=True, stop=True)
                shf = smalls.tile([P, 1], F32, tag=f"shf{cb}")
                nc.gpsimd.dma_start(out=shf, in_=shift_d[b, cb])
                # cst = shift - m*s - ukv (consumed as per-partition bias)
                cst = smalls.tile([P, 1], F32, tag=f"cst{cb}")
                nc.vector.tensor_mul(out=cst, in0=m_ap, in1=s_ap)
                nc.vector.tensor_sub(out=cst, in0=shf, in1=cst)
                nc.vector.tensor_sub(out=cst, in0=cst, in1=ukv)
                csts.append(cst)

            # ---- stage A: vtx[r, n]; M=12 matmuls run 4-at-a-time in
            # distinct PE column groups; one [128,1024] PSUM tile holds all
            # 8 chunks (strip j%4, column-half j//4) ----
            vaug = vaugs[b % 2]
            vtx1k = ps_vtx.tile([P, 1024], F32, tag="vtx1k")
            for half in range(2):
                for cb in range(CB):
                    for q in range(4):
                        j = 4 * half + q
                        nc.tensor.matmul(
                            vtx1k[32 * q:32 * q + R, half * 512:(half + 1) * 512],
                            lhsT=vss[cb],
                            rhs=xbt[(cb, j // 4)][:, (j % 4) * 512:(j % 4 + 1) * 512],
                            start=(cb == 0), stop=(cb == CB - 1),
                            tile_position=(0, 32 * q),
                            skip_group_check=True)
            for q in range(4):
                # strip q holds chunks q and q+4 -> vaug cols strided by 2048
                dst = bass.AP(
                    tensor=vaug.tensor,
                    offset=vaug.offset + q * 512,
                    ap=[vaug.ap[0], [2048, 2], [1, 512]],
                )
                nc.scalar.copy(out=dst, in_=vtx1k[32 * q:32 * q + R, :])

            kout = 0
            # ---- stage B: pm = u@vtx + s*x (PE); out = pm + cst (one
            # wide biased copy on ACT or DVE) ----
            for cb in range(CB):
                for h in range(2):
                    osb = outp.tile([P, 2048], F32, tag="osb")
                    for hh in range(2):
                        pm2 = ps_out.tile([P, 1024], F32, tag="pout")
                        for q2 in range(2):
                            j2 = h * 4 + hh * 2 + q2
                            pslice = pm2[:, q2 * 512:(q2 + 1) * 512]
                            nc.tensor.matmul(
                                pslice, lhsT=auglist[cb],
                                rhs=vaug[:, j2 * 512:(j2 + 1) * 512],
                                start=True, stop=False)
                            nc.tensor.matmul(
                                pslice, lhsT=diags[cb],
                                rhs=xbt[(cb, h)][:, (hh * 2 + q2) * 512:
                                                 (hh * 2 + q2 + 1) * 512],
                                start=False, stop=True)
                        o_ap = osb[:, hh * 1024:(hh + 1) * 1024]
                        ko = kout
                        kout += 1
                        if dve_out_mod > 0 and ko % dve_out_mod == 0:
                            nc.vector.tensor_scalar_add(
                                out=o_ap, in0=pm2, scalar1=csts[cb])
                        else:
                            nc.scalar.activation(
                                out=o_ap, in_=pm2,
                                func=mybir.ActivationFunctionType.Identity,
                                bias=csts[cb], scale=1.0)
                    nc.gpsimd.dma_start(
                        out=out_d[b, cb * P:(cb + 1) * P, h * 2048:(h + 1) * 2048],
                        in_=osb)
    # Bacc defers register allocation to its compile()/finalize() pipeline
    nc.finalize()
    return nc


def _host_prep(x, ccm_params):
    x = np.asarray(x, dtype=np.float32).reshape(B, C, HW).astype(np.float16)
    x = np.ascontiguousarray(x)
    cp = np.asarray(ccm_params, dtype=np.float32)
    u = cp[:, :C * R].reshape(B, C, R)
    v = cp[:, C * R:2 * C * R].reshape(B, C, R)
    shift = cp[:, 2 * C * R:].reshape(B, C)
    ut = np.ascontiguousarray(
        u.reshape(B, CB, P, R).transpose(0, 1, 3, 2)).astype(np.float16)
    vv = np.ascontiguousarray(v.reshape(B, CB, P, R))
    sh = np.ascontiguousarray(shift.reshape(B, CB, P, 1))
    gmask = np.zeros((P, 16), np.float32)
    gmask[np.arange(P), np.arange(P) // GPC] = 1.0
    gmaskT = np.ascontiguousarray(gmask.T)
    ident16 = np.eye(P, dtype=np.float16)
    in_maps = []
    for c in range(N_CORES):
        bs = slice(c * BPC, (c + 1) * BPC)
        in_maps.append({
            "x": x[bs], "ut": ut[bs], "v": vv[bs], "shift": sh[bs],
            "gmask": gmask, "gmaskT": gmaskT, "ident16": ident16,
        })
    return in_maps


def kernel(x, ccm_params, _trace=False, _dve_out_mod=4):
    in_maps = _host_prep(x, ccm_params)
    nc = build_nc(dve_out_mod=_dve_out_mod)
    res = run_bass_kernel_spmd(
        nc, in_maps, core_ids=list(range(N_CORES)), trace=_trace)
    out = np.concatenate([r["out"] for r in res.results], axis=0)
    out = out.reshape(B, C, H, W)
    if _trace:
        return out, res
    return out
